# revision 24
# baseline (speedup 1.0000x reference)
"""Bass TRN2 kernel for nn_LinearColumnwise: out = concat_rows(input) @ weight + bias.

Sharding: input [8, 2048, 4096] is row-sharded -- core i computes
out[i*2048:(i+1)*2048, :] = input[i] @ weight + bias locally; no collectives.

Per-core kernel: bf16 GEMM (host-cast + host-transposed lhsT), fp32 PSUM
accumulation, bias added on the Vector engine during PSUM->SBUF eviction.
Raw bass with hand-placed semaphores: every instruction carries at most one
wait plus one update (this toolchain rejects >2 sync commands per instruction).

Pipeline per core (M=2048, K=4096, N=4096), 32 output groups of 4 psum banks:
  SP   : pass-A stream = per-k-tile interleaved (w-slab0 tile, xt tile) pairs
         on one semaphore so the PE starts ~3us in and consumes at DMA rate;
         then double-buffered weight n-slabs + bias slices as usual
  PE   : nt=0 ms 0..7 run kt-OUTER across 8 psum banks (groups 0,1) overlapped
         with the input stream; groups 2..31 run kt-inner as usual
  DVE  : psum + bias -> SBUF staging (frees banks fast)
  ACT  : staging -> DRAM out
"""

import numpy as np
import ml_dtypes

P = 128
M = 2048          # rows per core
K = 4096          # contraction
N = 4096          # out features
KT = K // P       # 32 k-tiles
NT = 512          # psum-bank n tile
NNT = N // NT     # 8 n tiles
MS = M // P       # 16 m subtiles
N_CORES = 8
NGRP = 4 * NNT    # 32 output groups

_cached = None


def _build():
    import concourse.bass as bass
    import concourse.mybir as mybir

    f32 = mybir.dt.float32
    bf16 = mybir.dt.bfloat16
    ADD = mybir.AluOpType.add

    nc = bass.Bass()
    xt_d = nc.declare_dram_parameter("xt", [K, M], bf16, isOutput=False)
    # w host-split into two layouts (DMA traverses dst-partition-outer):
    #   w0   (kt, p, n): each pass-A k-tile is one linear 128KB block
    #   wrest (nt, p, kt, n): each n-slab is linear 4MB (32KB per partition)
    w0_d = nc.declare_dram_parameter("w0", [KT * P, NT], bf16, isOutput=False)
    wr_d = nc.declare_dram_parameter(
        "wrest", [(NNT - 1) * P * KT, NT], bf16, isOutput=False
    )
    b_d = nc.declare_dram_parameter("bias_bc", [P, N], f32, isOutput=False)
    out_d = nc.declare_dram_parameter("out", [M, N], f32, isOutput=True)

    xt_sb = nc.alloc_sbuf_tensor("xt_sb", [P, KT, M], bf16).ap()
    w_sb = [nc.alloc_sbuf_tensor(f"w_sb{b}", [P, KT, NT], bf16).ap() for b in range(2)]
    bias_sb = [nc.alloc_sbuf_tensor(f"bias{b}", [P, NT], f32).ap() for b in range(2)]
    stage = [nc.alloc_sbuf_tensor(f"stage{i}", [P, NT], f32).ap() for i in range(4)]
    ps = [nc.alloc_psum_tensor(f"ps{i}", [P, NT], f32).ap() for i in range(8)]

    xt_r = xt_d.rearrange("(kt p) m -> p kt m", p=P)
    w0_r = w0_d.rearrange("(kt p) n -> p kt n", p=P)
    wr_r = wr_d.rearrange("(nt p kt) n -> nt p kt n", nt=NNT - 1, p=P, kt=KT)

    # PE's wait value on w_sems[nt%2] once slab nt (>=1) must be present.
    # Slab 0 rides the pass-A in0 stream; odd slabs on w_sems[1], even on [0].
    def w_wait(nt):
        assert nt >= 1
        if nt % 2 == 0:
            return 16 * (nt // 2)
        return 16 * ((nt + 1) // 2)

    # pass-A chunking: first 4 k-tiles get their own sem (earliest possible
    # PE start), the rest pair up 2 k-tiles per sem. A single shared counter
    # is NOT safe: DMA sub-increments interleave across engines, so a global
    # count can hit 32(c+1) while a slice of pair c is still in flight.
    pa_chunks = [(c, c + 1) for c in range(4)] + [
        (c, c + 2) for c in range(4, KT, 2)
    ]

    def chunk_of(c):
        for j, (lo, hi) in enumerate(pa_chunks):
            if lo <= c < hi:
                return j, lo, hi
        raise AssertionError

    with nc.Block() as block:
        pa_sems = [nc.semaphore(f"pa{j}").__enter__() for j in range(len(pa_chunks))]
        hi_sem = nc.semaphore("xthi").__enter__()
        w_sems = [nc.semaphore(f"wsem{b}").__enter__() for b in range(2)]
        b_sems = [nc.semaphore(f"bsem{b}").__enter__() for b in range(2)]
        pe_sem = nc.semaphore("pe_grp").__enter__()
        cp_sem = nc.semaphore("copied").__enter__()
        ev_sem = nc.semaphore("evict").__enter__()

        @block.sync
        def _(sp):
            # pass-A stream: (w0 k-tile, xt lo-half k-tile) pairs -- pass A
            # only reads m<1024, so the stream is 12.25MB and stays ahead of
            # the PE; the xt hi-halves follow and gate pass B via hi_sem
            for c in range(KT):
                j = chunk_of(c)[0]
                sp.dma_start(
                    out=w_sb[0][:, c, :], in_=w0_r[:, c, :]
                ).then_inc(pa_sems[j], 16)
                sp.dma_start(
                    out=xt_sb[:, c, 0 : M // 2], in_=xt_r[:, c, 0 : M // 2]
                ).then_inc(pa_sems[j], 16)
                if c == 0:
                    sp.dma_start(out=bias_sb[0][:], in_=b_d[:, 0:NT]).then_inc(
                        b_sems[0], 16
                    )
            for c in range(KT):
                sp.dma_start(
                    out=xt_sb[:, c, M // 2 : M], in_=xt_r[:, c, M // 2 : M]
                ).then_inc(hi_sem, 16)
            for nt in range(1, NNT):
                if nt >= 2:
                    # w buffer nt%2 reused: PE must be done with slab nt-2
                    sp.wait_ge(pe_sem, 4 * (nt - 1))
                sp.dma_start(
                    out=w_sb[nt % 2][:], in_=wr_r[nt - 1]
                ).then_inc(w_sems[nt % 2], 16)
                if nt >= 2:
                    # bias buffer nt%2 reused: DVE done with groups of nt-2
                    sp.wait_ge(cp_sem, 16 * (nt - 1))
                sp.dma_start(
                    out=bias_sb[nt % 2][:], in_=b_d[:, nt * NT : (nt + 1) * NT]
                ).then_inc(b_sems[nt % 2], 16)

        @block.tensor
        def _(te):
            # pass A: nt=0, m-subtiles 0..7, kt-outer across 8 banks
            # (groups 0 and 1), streaming behind the paired input DMAs
            for c in range(KT):
                j, lo, hi = chunk_of(c)
                if c == lo:
                    te.wait_ge(pa_sems[j], 32 * (hi - lo))
                for ms in range(8):
                    inst = te.matmul(
                        ps[ms][:],
                        xt_sb[:, c, ms * P : (ms + 1) * P],
                        w_sb[0][:, c, :],
                        start=(c == 0),
                        stop=(c == KT - 1),
                    )
                    if c == KT - 1 and ms in (3, 7):
                        inst.then_inc(pe_sem, 1)
            # pass B: nt=0 groups 2,3 (m-subtiles 8..15), kt-inner; xt resident
            te.wait_ge(hi_sem, 16 * KT)
            for g in range(2, 4):
                te.wait_ge(cp_sem, 4 * (g - 1))
                bank0 = (g % 2) * 4
                inst = None
                for kt in range(KT):
                    for ms in range(4):
                        m0 = (g * 4 + ms) * P
                        inst = te.matmul(
                            ps[bank0 + ms][:],
                            xt_sb[:, kt, m0 : m0 + P],
                            w_sb[0][:, kt, :],
                            start=(kt == 0),
                            stop=(kt == KT - 1),
                        )
                inst.then_inc(pe_sem, 1)
            # nt >= 1: kt-inner 4-bank groups, double-buffered w slabs.
            # Final group (g=31) runs ms-OUTER with per-bank pe increments so
            # eviction + out-DMA of early banks overlap the remaining matmuls.
            for nt in range(1, NNT):
                te.wait_ge(w_sems[nt % 2], w_wait(nt))
                for mq in range(4):
                    g = 4 * nt + mq
                    # bank set g%2 reused from group g-2: DVE copied it out
                    te.wait_ge(cp_sem, 4 * (g - 1))
                    bank0 = (g % 2) * 4
                    if g == NGRP - 1:
                        for ms in range(4):
                            m0 = (mq * 4 + ms) * P
                            inst = None
                            for kt in range(KT):
                                inst = te.matmul(
                                    ps[bank0 + ms][:],
                                    xt_sb[:, kt, m0 : m0 + P],
                                    w_sb[nt % 2][:, kt, :],
                                    start=(kt == 0),
                                    stop=(kt == KT - 1),
                                )
                            inst.then_inc(pe_sem, 1)
                        continue
                    inst = None
                    for kt in range(KT):
                        for ms in range(4):
                            m0 = (mq * 4 + ms) * P
                            inst = te.matmul(
                                ps[bank0 + ms][:],
                                xt_sb[:, kt, m0 : m0 + P],
                                w_sb[nt % 2][:, kt, :],
                                start=(kt == 0),
                                stop=(kt == KT - 1),
                            )
                    inst.then_inc(pe_sem, 1)

        @block.vector
        def _(ve):
            for g in range(NGRP):
                nt = g // 4
                if g % 4 == 0:
                    ve.wait_ge(b_sems[nt % 2], 16 * (nt // 2 + 1))
                if g < NGRP - 1:
                    ve.wait_ge(pe_sem, g + 1)
                if g >= 1:
                    # staging slots reused every group: out-DMAs of g-1 done
                    ve.wait_ge(ev_sem, 64 * g)
                b = (g % 2) * 4
                for ms in range(4):
                    if g == NGRP - 1:
                        # final group: banks land one by one (ms-outer on PE)
                        ve.wait_ge(pe_sem, NGRP + ms)
                    ve.tensor_tensor(
                        stage[ms][:], ps[b + ms][:], bias_sb[nt % 2][:], ADD
                    ).then_inc(cp_sem, 1)

        @block.scalar
        def _(act):
            for g in range(NGRP):
                nt, mq = divmod(g, 4)
                if g < NGRP - 1:
                    act.wait_ge(cp_sem, 4 * (g + 1))
                for ms in range(4):
                    if g == NGRP - 1:
                        # final group: copies land one by one
                        act.wait_ge(cp_sem, 4 * g + ms + 1)
                    m0 = (mq * 4 + ms) * P
                    act.dma_start(
                        out=out_d[m0 : m0 + P, nt * NT : (nt + 1) * NT],
                        in_=stage[ms][:],
                    ).then_inc(ev_sem, 16)
            act.wait_ge(ev_sem, 16 * 4 * NGRP)

    return nc


def _get_nc():
    global _cached
    if _cached is None:
        _cached = _build()
    return _cached


def _prep_core_input(x_core, w0, wrest, bias_bc):
    # [2048, 4096] f32 -> transposed bf16 lhsT
    xt = np.ascontiguousarray(x_core.T).astype(ml_dtypes.bfloat16)
    return {"xt": xt, "w0": w0, "wrest": wrest, "bias_bc": bias_bc}


def _make_in_maps(inputs):
    input, weight, bias = inputs["input"], inputs["weight"], inputs["bias"]
    assert input.shape == (N_CORES, M, K)
    # split w[k, n] (k = kt*P + p, n = nt*NT + j) into the two DMA-friendly
    # layouts the kernel expects (see _build)
    wq = weight.astype(ml_dtypes.bfloat16).reshape(KT, P, NNT, NT)
    w0 = np.ascontiguousarray(wq[:, :, 0, :]).reshape(KT * P, NT)
    wrest = np.ascontiguousarray(wq[:, :, 1:, :].transpose(2, 1, 0, 3)).reshape(
        (NNT - 1) * P * KT, NT
    )
    bias_bc = np.ascontiguousarray(
        np.broadcast_to(bias.astype(np.float32), (P, N))
    )
    return [_prep_core_input(input[i], w0, wrest, bias_bc) for i in range(N_CORES)]


def kernel(input, weight, bias):
    from concourse.bass_utils import run_bass_kernel_spmd

    nc = _get_nc()
    in_maps = _make_in_maps({"input": input, "weight": weight, "bias": bias})
    res = run_bass_kernel_spmd(nc, in_maps, list(range(N_CORES)))
    return np.concatenate([res.results[i]["out"] for i in range(N_CORES)], axis=0)


# revision 29
# speedup vs baseline: 1.0015x; 1.0015x over previous
"""Bass TRN2 kernel for nn_LinearColumnwise: out = concat_rows(input) @ weight + bias.

Sharding: input [8, 2048, 4096] is row-sharded -- core i computes
out[i*2048:(i+1)*2048, :] = input[i] @ weight + bias locally; no collectives.

Per-core kernel: bf16 GEMM (host-cast + host-transposed lhsT), fp32 PSUM
accumulation, bias added on the Vector engine during PSUM->SBUF eviction.
Raw bass with hand-placed semaphores: every instruction carries at most one
wait plus one update (this toolchain rejects >2 sync commands per instruction).

Pipeline per core (M=2048, K=4096, N=4096), 32 output groups of 4 psum banks:
  SP   : pass-A stream = per-k-tile interleaved (w-slab0 tile, xt tile) pairs
         on one semaphore so the PE starts ~3us in and consumes at DMA rate;
         then double-buffered weight n-slabs + bias slices as usual
  PE   : nt=0 ms 0..7 run kt-OUTER across 8 psum banks (groups 0,1) overlapped
         with the input stream; groups 2..31 run kt-inner as usual
  DVE  : psum + bias -> SBUF staging (frees banks fast)
  ACT  : staging -> DRAM out
"""

import numpy as np
import ml_dtypes

P = 128
M = 2048          # rows per core
K = 4096          # contraction
N = 4096          # out features
KT = K // P       # 32 k-tiles
NT = 512          # psum-bank n tile
NNT = N // NT     # 8 n tiles
MS = M // P       # 16 m subtiles
N_CORES = 8
NGRP = 4 * NNT    # 32 output groups

_cached = None


def _build():
    import concourse.bass as bass
    import concourse.mybir as mybir

    f32 = mybir.dt.float32
    bf16 = mybir.dt.bfloat16
    ADD = mybir.AluOpType.add

    nc = bass.Bass()
    # xt host-split into m-halves so pass-A (lo) and deferred (hi) k-tile
    # transfers each read a linear 256KB DRAM block
    xlo_d = nc.declare_dram_parameter("xt_lo", [K, M // 2], bf16, isOutput=False)
    xhi_d = nc.declare_dram_parameter("xt_hi", [K, M // 2], bf16, isOutput=False)
    # w host-split into two layouts (DMA traverses dst-partition-outer):
    #   w0   (kt, p, n): each pass-A k-tile is one linear 128KB block
    #   wrest (nt, p, kt, n): each n-slab is linear 4MB (32KB per partition)
    w0_d = nc.declare_dram_parameter("w0", [KT * P, NT], bf16, isOutput=False)
    wr_d = nc.declare_dram_parameter(
        "wrest", [(NNT - 1) * P * KT, NT], bf16, isOutput=False
    )
    b_d = nc.declare_dram_parameter("bias_bc", [P, N], f32, isOutput=False)
    out_d = nc.declare_dram_parameter("out", [M, N], f32, isOutput=True)

    xt_sb = nc.alloc_sbuf_tensor("xt_sb", [P, KT, M], bf16).ap()
    w_sb = [nc.alloc_sbuf_tensor(f"w_sb{b}", [P, KT, NT], bf16).ap() for b in range(2)]
    bias_sb = [nc.alloc_sbuf_tensor(f"bias{b}", [P, NT], f32).ap() for b in range(2)]
    stage = [nc.alloc_sbuf_tensor(f"stage{i}", [P, NT], f32).ap() for i in range(4)]
    ps = [nc.alloc_psum_tensor(f"ps{i}", [P, NT], f32).ap() for i in range(8)]

    xlo_r = xlo_d.rearrange("(kt p) m -> p kt m", p=P)
    xhi_r = xhi_d.rearrange("(kt p) m -> p kt m", p=P)
    w0_r = w0_d.rearrange("(kt p) n -> p kt n", p=P)
    wr_r = wr_d.rearrange("(nt p kt) n -> nt p kt n", nt=NNT - 1, p=P, kt=KT)

    # PE's wait value on w_sems[nt%2] once slab nt (>=1) must be present.
    # Slab 0 rides the pass-A in0 stream; odd slabs on w_sems[1], even on [0].
    def w_wait(nt):
        assert nt >= 1
        if nt % 2 == 0:
            return 16 * (nt // 2)
        return 16 * ((nt + 1) // 2)

    # pass-A chunking: first 4 k-tiles get their own sem (earliest possible
    # PE start), the rest pair up 2 k-tiles per sem. A single shared counter
    # is NOT safe: DMA sub-increments interleave across engines, so a global
    # count can hit 32(c+1) while a slice of pair c is still in flight.
    pa_chunks = [(c, c + 1) for c in range(4)] + [
        (c, c + 2) for c in range(4, KT, 2)
    ]

    def chunk_of(c):
        for j, (lo, hi) in enumerate(pa_chunks):
            if lo <= c < hi:
                return j, lo, hi
        raise AssertionError

    with nc.Block() as block:
        pa_sems = [nc.semaphore(f"pa{j}").__enter__() for j in range(len(pa_chunks))]
        hi_sem = nc.semaphore("xthi").__enter__()
        w_sems = [nc.semaphore(f"wsem{b}").__enter__() for b in range(2)]
        b_sems = [nc.semaphore(f"bsem{b}").__enter__() for b in range(2)]
        pe_sem = nc.semaphore("pe_grp").__enter__()
        cp_sem = nc.semaphore("copied").__enter__()
        ev_sem = nc.semaphore("evict").__enter__()

        @block.sync
        def _(sp):
            # pass-A stream: (w0 k-tile, xt lo-half k-tile) pairs -- pass A
            # only reads m<1024, so the stream is 12.25MB and stays ahead of
            # the PE; the xt hi-halves follow and gate pass B via hi_sem
            for c in range(KT):
                j = chunk_of(c)[0]
                sp.dma_start(
                    out=w_sb[0][:, c, :], in_=w0_r[:, c, :]
                ).then_inc(pa_sems[j], 16)
                sp.dma_start(
                    out=xt_sb[:, c, 0 : M // 2], in_=xlo_r[:, c, :]
                ).then_inc(pa_sems[j], 16)
                if c == 0:
                    sp.dma_start(out=bias_sb[0][:], in_=b_d[:, 0:NT]).then_inc(
                        b_sems[0], 16
                    )
            for c in range(KT):
                sp.dma_start(
                    out=xt_sb[:, c, M // 2 : M], in_=xhi_r[:, c, :]
                ).then_inc(hi_sem, 16)
            for nt in range(1, NNT):
                if nt >= 2:
                    # w buffer nt%2 reused: PE must be done with slab nt-2
                    sp.wait_ge(pe_sem, 4 * (nt - 1))
                sp.dma_start(
                    out=w_sb[nt % 2][:], in_=wr_r[nt - 1]
                ).then_inc(w_sems[nt % 2], 16)
                if nt >= 2:
                    # bias buffer nt%2 reused: DVE done with groups of nt-2
                    sp.wait_ge(cp_sem, 16 * (nt - 1))
                sp.dma_start(
                    out=bias_sb[nt % 2][:], in_=b_d[:, nt * NT : (nt + 1) * NT]
                ).then_inc(b_sems[nt % 2], 16)

        @block.tensor
        def _(te):
            # pass A: nt=0, m-subtiles 0..7, kt-outer across 8 banks
            # (groups 0 and 1), streaming behind the paired input DMAs
            for c in range(KT):
                j, lo, hi = chunk_of(c)
                if c == lo:
                    te.wait_ge(pa_sems[j], 32 * (hi - lo))
                for ms in range(8):
                    inst = te.matmul(
                        ps[ms][:],
                        xt_sb[:, c, ms * P : (ms + 1) * P],
                        w_sb[0][:, c, :],
                        start=(c == 0),
                        stop=(c == KT - 1),
                    )
                    if c == KT - 1 and ms in (3, 7):
                        inst.then_inc(pe_sem, 1)
            # pass B: nt=0 groups 2,3 (m-subtiles 8..15), kt-inner; xt resident
            te.wait_ge(hi_sem, 16 * KT)
            for g in range(2, 4):
                te.wait_ge(cp_sem, 4 * (g - 1))
                bank0 = (g % 2) * 4
                inst = None
                for kt in range(KT):
                    for ms in range(4):
                        m0 = (g * 4 + ms) * P
                        inst = te.matmul(
                            ps[bank0 + ms][:],
                            xt_sb[:, kt, m0 : m0 + P],
                            w_sb[0][:, kt, :],
                            start=(kt == 0),
                            stop=(kt == KT - 1),
                        )
                inst.then_inc(pe_sem, 1)
            # nt >= 1: kt-inner 4-bank groups, double-buffered w slabs.
            # Final group (g=31) runs ms-OUTER with per-bank pe increments so
            # eviction + out-DMA of early banks overlap the remaining matmuls.
            for nt in range(1, NNT):
                te.wait_ge(w_sems[nt % 2], w_wait(nt))
                for mq in range(4):
                    g = 4 * nt + mq
                    # bank set g%2 reused from group g-2: DVE copied it out
                    te.wait_ge(cp_sem, 4 * (g - 1))
                    bank0 = (g % 2) * 4
                    if g == NGRP - 1:
                        for ms in range(4):
                            m0 = (mq * 4 + ms) * P
                            inst = None
                            for kt in range(KT):
                                inst = te.matmul(
                                    ps[bank0 + ms][:],
                                    xt_sb[:, kt, m0 : m0 + P],
                                    w_sb[nt % 2][:, kt, :],
                                    start=(kt == 0),
                                    stop=(kt == KT - 1),
                                )
                            inst.then_inc(pe_sem, 1)
                        continue
                    inst = None
                    for kt in range(KT):
                        for ms in range(4):
                            m0 = (mq * 4 + ms) * P
                            inst = te.matmul(
                                ps[bank0 + ms][:],
                                xt_sb[:, kt, m0 : m0 + P],
                                w_sb[nt % 2][:, kt, :],
                                start=(kt == 0),
                                stop=(kt == KT - 1),
                            )
                    inst.then_inc(pe_sem, 1)

        @block.vector
        def _(ve):
            for g in range(NGRP):
                nt = g // 4
                if g % 4 == 0:
                    ve.wait_ge(b_sems[nt % 2], 16 * (nt // 2 + 1))
                if g < NGRP - 1:
                    ve.wait_ge(pe_sem, g + 1)
                if g >= 1:
                    # staging slots reused every group: out-DMAs of g-1 done
                    ve.wait_ge(ev_sem, 64 * g)
                b = (g % 2) * 4
                for ms in range(4):
                    if g == NGRP - 1:
                        # final group: banks land one by one (ms-outer on PE)
                        ve.wait_ge(pe_sem, NGRP + ms)
                    ve.tensor_tensor(
                        stage[ms][:], ps[b + ms][:], bias_sb[nt % 2][:], ADD
                    ).then_inc(cp_sem, 1)

        @block.scalar
        def _(act):
            for g in range(NGRP):
                nt, mq = divmod(g, 4)
                if g < NGRP - 1:
                    act.wait_ge(cp_sem, 4 * (g + 1))
                for ms in range(4):
                    if g == NGRP - 1:
                        # final group: copies land one by one
                        act.wait_ge(cp_sem, 4 * g + ms + 1)
                    m0 = (mq * 4 + ms) * P
                    act.dma_start(
                        out=out_d[m0 : m0 + P, nt * NT : (nt + 1) * NT],
                        in_=stage[ms][:],
                    ).then_inc(ev_sem, 16)
            act.wait_ge(ev_sem, 16 * 4 * NGRP)

    return nc


def _get_nc():
    global _cached
    if _cached is None:
        _cached = _build()
    return _cached


def _prep_core_input(x_core, w0, wrest, bias_bc):
    # [2048, 4096] f32 -> transposed bf16 lhsT, split into m-halves
    xt_lo = np.ascontiguousarray(x_core[: M // 2].T).astype(ml_dtypes.bfloat16)
    xt_hi = np.ascontiguousarray(x_core[M // 2 :].T).astype(ml_dtypes.bfloat16)
    return {"xt_lo": xt_lo, "xt_hi": xt_hi, "w0": w0, "wrest": wrest, "bias_bc": bias_bc}


def _make_in_maps(inputs):
    input, weight, bias = inputs["input"], inputs["weight"], inputs["bias"]
    assert input.shape == (N_CORES, M, K)
    # split w[k, n] (k = kt*P + p, n = nt*NT + j) into the two DMA-friendly
    # layouts the kernel expects (see _build)
    wq = weight.astype(ml_dtypes.bfloat16).reshape(KT, P, NNT, NT)
    w0 = np.ascontiguousarray(wq[:, :, 0, :]).reshape(KT * P, NT)
    wrest = np.ascontiguousarray(wq[:, :, 1:, :].transpose(2, 1, 0, 3)).reshape(
        (NNT - 1) * P * KT, NT
    )
    bias_bc = np.ascontiguousarray(
        np.broadcast_to(bias.astype(np.float32), (P, N))
    )
    return [_prep_core_input(input[i], w0, wrest, bias_bc) for i in range(N_CORES)]


def kernel(input, weight, bias):
    from concourse.bass_utils import run_bass_kernel_spmd

    nc = _get_nc()
    in_maps = _make_in_maps({"input": input, "weight": weight, "bias": bias})
    res = run_bass_kernel_spmd(nc, in_maps, list(range(N_CORES)))
    return np.concatenate([res.results[i]["out"] for i in range(N_CORES)], axis=0)


# revision 36
# speedup vs baseline: 1.0120x; 1.0106x over previous
"""Bass TRN2 kernel for nn_LinearColumnwise: out = concat_rows(input) @ weight + bias.

Sharding: input [8, 2048, 4096] is row-sharded -- core i computes
out[i*2048:(i+1)*2048, :] = input[i] @ weight + bias locally; no collectives.

Per-core kernel: bf16 GEMM (host-cast + host-transposed lhsT), fp32 PSUM
accumulation, bias added on the Vector engine during PSUM->SBUF eviction.
Raw bass with hand-placed semaphores: every instruction carries at most one
wait plus one update (this toolchain rejects >2 sync commands per instruction).

Pipeline per core (M=2048, K=4096, N=4096), 32 output groups of 4 psum banks:
  SP   : pass-A stream = per-k-tile interleaved (w-slab0 tile, xt tile) pairs
         on one semaphore so the PE starts ~3us in and consumes at DMA rate;
         then double-buffered weight n-slabs + bias slices as usual
  PE   : nt=0 ms 0..7 run kt-OUTER across 8 psum banks (groups 0,1) overlapped
         with the input stream; groups 2..31 run kt-inner as usual
  DVE  : psum + bias -> SBUF staging (frees banks fast)
  ACT  : staging -> DRAM out
"""

import numpy as np
import ml_dtypes

P = 128
M = 2048          # rows per core
K = 4096          # contraction
N = 4096          # out features
KT = K // P       # 32 k-tiles
NT = 512          # psum-bank n tile
NNT = N // NT     # 8 n tiles
MS = M // P       # 16 m subtiles
N_CORES = 8
NGRP = 4 * NNT    # 32 output groups

_cached = None


def _build():
    import concourse.bass as bass
    import concourse.mybir as mybir

    f32 = mybir.dt.float32
    bf16 = mybir.dt.bfloat16
    ADD = mybir.AluOpType.add

    nc = bass.Bass()
    xt_d = nc.declare_dram_parameter("xt", [K, M], bf16, isOutput=False)
    # w host-split into two layouts (DMA traverses dst-partition-outer):
    #   w0   (kt, p, n): each pass-A k-tile is one linear 128KB block
    #   wrest (nt, p, kt, n): each n-slab is linear 4MB (32KB per partition)
    w0_d = nc.declare_dram_parameter("w0", [KT * P, NT], bf16, isOutput=False)
    wr_d = nc.declare_dram_parameter(
        "wrest", [(NNT - 1) * P * KT, NT], bf16, isOutput=False
    )
    b_d = nc.declare_dram_parameter("bias_bc", [P, N], f32, isOutput=False)
    out_d = nc.declare_dram_parameter("out", [M, N], f32, isOutput=True)

    xt_sb = nc.alloc_sbuf_tensor("xt_sb", [P, KT, M], bf16).ap()
    w_sb = [nc.alloc_sbuf_tensor(f"w_sb{b}", [P, KT, NT], bf16).ap() for b in range(2)]
    bias_sb = [nc.alloc_sbuf_tensor(f"bias{b}", [P, NT], f32).ap() for b in range(2)]
    stage = [nc.alloc_sbuf_tensor(f"stage{i}", [P, NT], f32).ap() for i in range(4)]
    ps = [nc.alloc_psum_tensor(f"ps{i}", [P, NT], f32).ap() for i in range(8)]

    xt_r = xt_d.rearrange("(kt p) m -> p kt m", p=P)
    w0_r = w0_d.rearrange("(kt p) n -> p kt n", p=P)
    wr_r = wr_d.rearrange("(nt p kt) n -> nt p kt n", nt=NNT - 1, p=P, kt=KT)

    # PE's wait value on w_sems[nt%2] once slab nt (>=1) must be present.
    # Slab 0 rides the pass-A in0 stream; odd slabs on w_sems[1], even on [0].
    def w_wait(nt):
        assert nt >= 1
        if nt % 2 == 0:
            return 16 * (nt // 2)
        return 16 * ((nt + 1) // 2)

    # pass-A chunking: one (w0, xt) transfer PAIR per chunk -- per-transfer
    # ring overhead is ~1us, so small per-k-tile transfers starve the PE.
    # First two chunks are 2 k-tiles (quick PE start), rest are 4. Each chunk
    # has its own sem: a single shared counter is NOT safe (DMA sub-
    # increments interleave across engines).
    pa_chunks = [(0, 2), (2, 4)] + [(c, c + 4) for c in range(4, KT, 4)]

    def chunk_of(c):
        for j, (lo, hi) in enumerate(pa_chunks):
            if lo <= c < hi:
                return j, lo, hi
        raise AssertionError

    with nc.Block() as block:
        pa_sems = [nc.semaphore(f"pa{j}").__enter__() for j in range(len(pa_chunks))]
        w_sems = [nc.semaphore(f"wsem{b}").__enter__() for b in range(2)]
        b_sems = [nc.semaphore(f"bsem{b}").__enter__() for b in range(2)]
        pe_sem = nc.semaphore("pe_grp").__enter__()
        cp_sem = nc.semaphore("copied").__enter__()
        ev_sem = nc.semaphore("evict").__enter__()

        @block.sync
        def _(sp):
            # pass-A stream: one (w0, xt) linear transfer pair per chunk
            for j, (lo, hi) in enumerate(pa_chunks):
                sp.dma_start(
                    out=w_sb[0][:, lo:hi, :], in_=w0_r[:, lo:hi, :]
                ).then_inc(pa_sems[j], 16)
                sp.dma_start(
                    out=xt_sb[:, lo:hi, :], in_=xt_r[:, lo:hi, :]
                ).then_inc(pa_sems[j], 16)
                if j == 0:
                    sp.dma_start(out=bias_sb[0][:], in_=b_d[:, 0:NT]).then_inc(
                        b_sems[0], 16
                    )
            for nt in range(1, NNT):
                if nt >= 2:
                    # w buffer nt%2 reused: PE must be done with slab nt-2
                    sp.wait_ge(pe_sem, 4 * (nt - 1))
                sp.dma_start(
                    out=w_sb[nt % 2][:], in_=wr_r[nt - 1]
                ).then_inc(w_sems[nt % 2], 16)
                if nt >= 2:
                    # bias buffer nt%2 reused: DVE done with groups of nt-2
                    sp.wait_ge(cp_sem, 16 * (nt - 1))
                sp.dma_start(
                    out=bias_sb[nt % 2][:], in_=b_d[:, nt * NT : (nt + 1) * NT]
                ).then_inc(b_sems[nt % 2], 16)

        @block.tensor
        def _(te):
            # pass A: nt=0, m-subtiles 0..7, kt-outer across 8 banks
            # (groups 0 and 1), streaming behind the paired input DMAs
            for c in range(KT):
                j, lo, hi = chunk_of(c)
                if c == lo:
                    te.wait_ge(pa_sems[j], 32)
                for ms in range(8):
                    inst = te.matmul(
                        ps[ms][:],
                        xt_sb[:, c, ms * P : (ms + 1) * P],
                        w_sb[0][:, c, :],
                        start=(c == 0),
                        stop=(c == KT - 1),
                    )
                    if c == KT - 1 and ms in (3, 7):
                        inst.then_inc(pe_sem, 1)
            # pass B: nt=0 groups 2,3 (m-subtiles 8..15), kt-inner; xt resident
            for g in range(2, 4):
                te.wait_ge(cp_sem, 4 * (g - 1))
                bank0 = (g % 2) * 4
                inst = None
                for kt in range(KT):
                    for ms in range(4):
                        m0 = (g * 4 + ms) * P
                        inst = te.matmul(
                            ps[bank0 + ms][:],
                            xt_sb[:, kt, m0 : m0 + P],
                            w_sb[0][:, kt, :],
                            start=(kt == 0),
                            stop=(kt == KT - 1),
                        )
                inst.then_inc(pe_sem, 1)
            # nt >= 1: kt-inner 4-bank groups, double-buffered w slabs.
            # Final group (g=31) runs ms-OUTER with per-bank pe increments so
            # eviction + out-DMA of early banks overlap the remaining matmuls.
            for nt in range(1, NNT):
                te.wait_ge(w_sems[nt % 2], w_wait(nt))
                for mq in range(4):
                    g = 4 * nt + mq
                    # bank set g%2 reused from group g-2: DVE copied it out
                    te.wait_ge(cp_sem, 4 * (g - 1))
                    bank0 = (g % 2) * 4
                    if g == NGRP - 1:
                        for ms in range(4):
                            m0 = (mq * 4 + ms) * P
                            inst = None
                            for kt in range(KT):
                                inst = te.matmul(
                                    ps[bank0 + ms][:],
                                    xt_sb[:, kt, m0 : m0 + P],
                                    w_sb[nt % 2][:, kt, :],
                                    start=(kt == 0),
                                    stop=(kt == KT - 1),
                                )
                            inst.then_inc(pe_sem, 1)
                        continue
                    inst = None
                    for kt in range(KT):
                        for ms in range(4):
                            m0 = (mq * 4 + ms) * P
                            inst = te.matmul(
                                ps[bank0 + ms][:],
                                xt_sb[:, kt, m0 : m0 + P],
                                w_sb[nt % 2][:, kt, :],
                                start=(kt == 0),
                                stop=(kt == KT - 1),
                            )
                    inst.then_inc(pe_sem, 1)

        @block.vector
        def _(ve):
            for g in range(NGRP):
                nt = g // 4
                if g % 4 == 0:
                    ve.wait_ge(b_sems[nt % 2], 16 * (nt // 2 + 1))
                if g < NGRP - 1:
                    ve.wait_ge(pe_sem, g + 1)
                if g >= 1:
                    # staging slots reused every group: out-DMAs of g-1 done
                    ve.wait_ge(ev_sem, 64 * g)
                b = (g % 2) * 4
                for ms in range(4):
                    if g == NGRP - 1:
                        # final group: banks land one by one (ms-outer on PE)
                        ve.wait_ge(pe_sem, NGRP + ms)
                    ve.tensor_tensor(
                        stage[ms][:], ps[b + ms][:], bias_sb[nt % 2][:], ADD
                    ).then_inc(cp_sem, 1)

        @block.scalar
        def _(act):
            for g in range(NGRP):
                nt, mq = divmod(g, 4)
                if g < NGRP - 1:
                    act.wait_ge(cp_sem, 4 * (g + 1))
                for ms in range(4):
                    if g == NGRP - 1:
                        # final group: copies land one by one
                        act.wait_ge(cp_sem, 4 * g + ms + 1)
                    m0 = (mq * 4 + ms) * P
                    act.dma_start(
                        out=out_d[m0 : m0 + P, nt * NT : (nt + 1) * NT],
                        in_=stage[ms][:],
                    ).then_inc(ev_sem, 16)
            act.wait_ge(ev_sem, 16 * 4 * NGRP)

    return nc


def _get_nc():
    global _cached
    if _cached is None:
        _cached = _build()
    return _cached


def _prep_core_input(x_core, w0, wrest, bias_bc):
    # [2048, 4096] f32 -> transposed bf16 lhsT
    xt = np.ascontiguousarray(x_core.T).astype(ml_dtypes.bfloat16)
    return {"xt": xt, "w0": w0, "wrest": wrest, "bias_bc": bias_bc}


def _make_in_maps(inputs):
    input, weight, bias = inputs["input"], inputs["weight"], inputs["bias"]
    assert input.shape == (N_CORES, M, K)
    # split w[k, n] (k = kt*P + p, n = nt*NT + j) into the two DMA-friendly
    # layouts the kernel expects (see _build)
    wq = weight.astype(ml_dtypes.bfloat16).reshape(KT, P, NNT, NT)
    w0 = np.ascontiguousarray(wq[:, :, 0, :]).reshape(KT * P, NT)
    wrest = np.ascontiguousarray(wq[:, :, 1:, :].transpose(2, 1, 0, 3)).reshape(
        (NNT - 1) * P * KT, NT
    )
    bias_bc = np.ascontiguousarray(
        np.broadcast_to(bias.astype(np.float32), (P, N))
    )
    return [_prep_core_input(input[i], w0, wrest, bias_bc) for i in range(N_CORES)]


def kernel(input, weight, bias):
    from concourse.bass_utils import run_bass_kernel_spmd

    nc = _get_nc()
    in_maps = _make_in_maps({"input": input, "weight": weight, "bias": bias})
    res = run_bass_kernel_spmd(nc, in_maps, list(range(N_CORES)))
    return np.concatenate([res.results[i]["out"] for i in range(N_CORES)], axis=0)
